# revision 1
# baseline (speedup 1.0000x reference)
"""Trainium2 Bass kernel for nn_AdaptiveEncoderCls_so (retrieval_knn).

Strategy: data-parallel over batch B=32 across 8 NeuronCores (4 batch
elements per core).  The inherently sequential index computations (furthest
point sampling chains, exact top-k neighbor selection, and the gathers that
depend on them) run on host in numpy; all dense math (adaptive embeddings
via ACT exp/sin, KNN feature normalization, aggregation, pooling, gelu)
runs on-device as 5 SPMD Bass phases (initial embedding + one per stage).
Cross-batch statistics (per-k stds, global gstd) are computed between
phases on host, and enter the device kernels as small input tensors so the
compiled NEFFs are input-independent and cached across calls.
"""

import math
import sys

import numpy as np
import ml_dtypes

sys.path.insert(0, "/opt/trn_rl_repo")

import concourse.bass as bass  # noqa: E402
from concourse.bacc import Bacc  # noqa: E402
import concourse.mybir as mybir  # noqa: E402
from concourse import bass_utils  # noqa: E402
from concourse import bass_isa  # noqa: E402
from concourse.tile import TileContext  # noqa: E402

F32 = mybir.dt.float32
BF16 = mybir.dt.bfloat16
ALU = mybir.AluOpType
ACTF = mybir.ActivationFunctionType

NCORES = 8
B, N, K = 32, 2048, 32
BL = B // NCORES  # batch elements per core
INIT_DIM = 32
SIGMA, BASELINE, SCALING, EPS = 0.26, 0.1, 10.0, 1e-6
STAGES = [(1024, 64), (512, 128), (256, 256), (128, 512)]  # (S, out_dim)
KT_BY_OD = {64: 32, 128: 32, 256: 32, 512: 16}

_BF = ml_dtypes.bfloat16
LAST_EXEC_NS = 0  # accumulated device-exec time of spmd calls (ns)
TRACE = False     # when True, capture ntff profiles (real exec_time_ns)
PROFILES = []     # (tag, tmpdir, exec_time_ns) per phase when TRACE


# ----------------------------------------------------------------------------
# host-side index math (numpy, float32 to mirror the reference's fp behavior)
# ----------------------------------------------------------------------------

def _fps(xyz, npoint):
    """Furthest point sampling, pointnet2 semantics (start at index 0)."""
    Bb, Nn, _ = xyz.shape
    dist = np.full((Bb, Nn), np.inf, np.float32)
    far = np.zeros(Bb, np.int64)
    idxs = np.empty((Bb, npoint), np.int64)
    ar = np.arange(Bb)
    buf = np.empty_like(xyz)
    d = np.empty((Bb, Nn), np.float32)
    for i in range(npoint):
        idxs[:, i] = far
        c = xyz[ar, far]  # [B,3]
        np.subtract(xyz, c[:, None, :], out=buf)
        np.multiply(buf, buf, out=buf)
        buf.sum(-1, out=d)
        np.minimum(dist, d, out=dist)
        far = dist.argmax(-1)
    return idxs


def _knn_idx(xyz_s, xyz):
    """Indices of K nearest points of xyz for each row of xyz_s (sorted by
    ascending distance, ties broken by lower index — matches lax.top_k on
    the negated squared distance)."""
    sq = -2.0 * np.matmul(xyz_s, xyz.transpose(0, 2, 1))
    sq += (xyz_s ** 2).sum(-1, dtype=np.float32)[:, :, None]
    sq += (xyz ** 2).sum(-1, dtype=np.float32)[:, None, :]
    # argpartition + stable (value, index) sort of the candidate set ==
    # full stable argsort restricted to the K smallest (PAD absorbs any
    # fp ties straddling the partition boundary)
    M = min(K + 16, sq.shape[-1])
    if M >= sq.shape[-1]:
        return np.argsort(sq, axis=-1, kind="stable")[:, :, :K]
    part = np.argpartition(sq, M, axis=-1)[:, :, :M]
    vals = np.take_along_axis(sq, part, axis=-1)
    order = np.lexsort((part, vals), axis=-1)[:, :, :K]
    return np.take_along_axis(part, order, axis=-1)


def _emb_params(x_b_m_3, out_dim):
    fd = math.ceil(out_dim / 3)
    fn = fd * 3
    out_idx = np.floor(np.linspace(0, fn - 1, out_dim)).astype(np.int64)
    fv = np.linspace(-1.0, 1.0, fd + 2)[1:-1].astype(np.float32)
    gstd = float(np.mean(np.std(x_b_m_3, axis=1, ddof=1)))
    asig = SIGMA * (1.0 + gstd)
    blend = float(1.0 / (1.0 + np.exp(-(gstd - BASELINE) * SCALING)))
    return fd, fn, out_idx, fv, float(asig), blend


def _runs(out_idx):
    """Decompose the (strictly increasing) out_idx selection into contiguous
    copy runs: list of (src_start, dst_start, length)."""
    runs = []
    s = 0
    n = len(out_idx)
    while s < n:
        e = s
        while e + 1 < n and out_idx[e + 1] == out_idx[e] + 1:
            e += 1
        runs.append((int(out_idx[s]), s, e - s + 1))
        s = e + 1
    return runs


def _bcast(a, b):
    """Broadcast two same-ndim APs against each other (0-stride expansion)."""
    return bass.broadcast_tensor_aps(a, b)


# ----------------------------------------------------------------------------
# device graphs
# ----------------------------------------------------------------------------

def build_phase0():
    """feat0 = adaptive_embedding(xyz, 32) for this core's BL*N points."""
    fd, fn = 11, 33
    out_idx = np.floor(np.linspace(0, fn - 1, INIT_DIM)).astype(np.int64)
    runs = _runs(out_idx)

    nc = Bacc()
    xyz = nc.dram_tensor("xyz", [BL * N, 3], F32, kind="ExternalInput")
    fv = nc.dram_tensor("fv", [128, 3 * fd], F32, kind="ExternalInput")
    sc = nc.dram_tensor("sc", [128, 4], F32, kind="ExternalInput")
    out = nc.dram_tensor("out", [BL * N, INIT_DIM], F32, kind="ExternalOutput")

    PTS = BL * N // 128  # points per partition

    with TileContext(nc) as tc:
        with tc.tile_pool(name="p0", bufs=1) as pool:
            fvt = pool.tile([128, 3 * fd], F32)
            nc.sync.dma_start(fvt[:], fv[:])
            sct = pool.tile([128, 4], F32)
            nc.sync.dma_start(sct[:], sc[:])
            xt = pool.tile([128, PTS, 3], F32)
            nc.sync.dma_start(xt[:], xyz.rearrange("(p n) c -> p n c", p=128))

            diff = pool.tile([128, PTS, 3, fd], F32)
            a4 = xt[:].unsqueeze(3)
            b4 = fvt[:].rearrange("p (c j) -> p c j", c=3).unsqueeze(1)
            a4, b4 = _bcast(a4, b4)
            nc.vector.tensor_tensor(diff[:], a4, b4, ALU.subtract)

            dflat = diff[:].rearrange("p n c j -> p (n c j)")
            sq = pool.tile([128, PTS * fn], F32)
            nc.scalar.activation(sq[:], dflat, ACTF.Square, scale=sct[:, 0:1])
            ex = pool.tile([128, PTS * fn], F32)
            nc.scalar.activation(ex[:], sq[:], ACTF.Exp, scale=-0.5)
            co = pool.tile([128, PTS * fn], F32)
            nc.scalar.activation(
                co[:], dflat, ACTF.Sin, bias=sct[:, 2:3], scale=sct[:, 0:1]
            )
            # comb = blend*ex + (1-blend)*co = (ex - co)*blend + co
            dmc = pool.tile([128, PTS * fn], F32)
            nc.vector.tensor_tensor(dmc[:], ex[:], co[:], ALU.subtract)
            comb = pool.tile([128, PTS, fn], F32)
            nc.vector.scalar_tensor_tensor(
                comb[:].rearrange("p n j -> p (n j)"),
                dmc[:], sct[:, 1:2], co[:], ALU.mult, ALU.add,
            )

            feat = pool.tile([128, PTS, INIT_DIM], F32)
            for (srcs, dsts, ln) in runs:
                nc.vector.tensor_copy(
                    feat[:, :, dsts:dsts + ln], comb[:, :, srcs:srcs + ln]
                )
            nc.sync.dma_start(out.rearrange("(p n) c -> p n c", p=128), feat[:])
    nc.finalize()
    return nc


def build_stage(S, OD):
    """One encoder stage for this core's BL batch elements.

    Inputs (per core):
      araw [BL*S, K, C]  raw gathered neighbor features
      fs   [BL*S, C]     features of the sampled points themselves
      xn   [BL*S, K, 3]  normalized xyz_knn (host-normalized, xyz-side)
      fv   [128, 3*fd]   feature grid values (replicated rows)
      sc   [128, 4]      [inv_asig, blend, 0, 0] (replicated rows)
      isg  [128, K]      1/sigma_feat per neighbor rank (replicated rows)
    Outputs:
      outf [BL*S, OD]    gelu'd stage features (host gathers these next)
      outp [BL, 2*OD]    per-batch [max_S, mean_S] stage result piece
    """
    C = OD // 2
    fd = math.ceil(OD / 3)
    fn = 3 * fd
    out_idx = np.floor(np.linspace(0, fn - 1, OD)).astype(np.int64)
    runs = _runs(out_idx)
    KT = KT_BY_OD[OD]
    NKT = K // KT
    ROWS = BL * S
    TILES = ROWS // 128
    TPB = TILES // BL  # 128-row tiles per batch element

    nc = Bacc()
    araw = nc.dram_tensor("araw", [ROWS, K, C], BF16, kind="ExternalInput")
    fs = nc.dram_tensor("fs", [ROWS, C], BF16, kind="ExternalInput")
    xn = nc.dram_tensor("xn", [ROWS, K, 3], BF16, kind="ExternalInput")
    fv = nc.dram_tensor("fv", [128, 3 * fd], F32, kind="ExternalInput")
    sc = nc.dram_tensor("sc", [128, 4], F32, kind="ExternalInput")
    isg = nc.dram_tensor("isg", [128, K], F32, kind="ExternalInput")
    outf = nc.dram_tensor("outf", [ROWS, OD], BF16, kind="ExternalOutput")
    outp = nc.dram_tensor("outp", [BL, 2 * OD], F32, kind="ExternalOutput")

    with TileContext(nc) as tc:
        with tc.tile_pool(name="cst", bufs=1) as cpool, \
             tc.tile_pool(name="wrk", bufs=2) as pool, \
             tc.tile_pool(name="acc", bufs=1) as apool:
            fvt = cpool.tile([128, 3 * fd], F32)
            nc.sync.dma_start(fvt[:], fv[:])
            sct = cpool.tile([128, 4], F32)
            nc.sync.dma_start(sct[:], sc[:])
            isgt = cpool.tile([128, K], BF16)
            nc.gpsimd.dma_start(isgt[:], isg[:])

            accs = []
            accm = []
            for b in range(BL):
                ts_ = apool.tile([128, OD], F32, name=f"accs{b}", tag=f"accs{b}")
                tm_ = apool.tile([128, OD], F32, name=f"accm{b}", tag=f"accm{b}")
                accs.append(ts_)
                accm.append(tm_)

            for ti in range(TILES):
                b = ti // TPB
                first = (ti % TPB) == 0
                r0 = ti * 128

                fst = pool.tile([128, C], BF16, tag="fst", bufs=2)
                nc.sync.dma_start(fst[:], fs[r0:r0 + 128])
                xnt = pool.tile([128, K, 3], F32, tag="xnt", bufs=2)
                nc.gpsimd.dma_start(xnt[:], xn[r0:r0 + 128])
                fsb = fst[:].unsqueeze(1)

                wsum = pool.tile([128, OD], F32, tag="wsum", bufs=2)
                wmax = pool.tile([128, OD], F32, tag="wmax", bufs=2)

                for kc in range(NKT):
                    ks = kc * KT
                    # A_left = (araw - fs) * isg for this k-chunk (right half
                    # of A is just fs broadcast — fused into the w add below)
                    artc = pool.tile([128, KT, C], BF16, tag="artc", bufs=2)
                    nc.sync.dma_start(artc[:], araw[r0:r0 + 128, ks:ks + KT, :])
                    Atc = pool.tile([128, KT, C], BF16, tag="Atc", bufs=1)
                    a3, b3 = _bcast(artc[:], fsb)
                    nc.vector.tensor_tensor(Atc[:], a3, b3, ALU.subtract)
                    ig3 = isgt[:, ks:ks + KT].unsqueeze(2)
                    a3, b3 = _bcast(Atc[:], ig3)
                    nc.vector.tensor_tensor(Atc[:], a3, b3, ALU.mult)

                    diff = pool.tile([128, KT, 3, fd], BF16, tag="diff", bufs=1)
                    a4 = xnt[:, ks:ks + KT, :].unsqueeze(3)
                    b4 = fvt[:].rearrange("p (c j) -> p c j", c=3).unsqueeze(1)
                    a4, b4 = _bcast(a4, b4)
                    nc.vector.tensor_tensor(diff[:], a4, b4, ALU.subtract)

                    dflat = diff[:].rearrange("p k c j -> p (k c j)")
                    sq = pool.tile([128, KT * fn], BF16, tag="sq", bufs=1)
                    nc.scalar.activation(sq[:], dflat, ACTF.Square,
                                         scale=sct[:, 0:1])
                    # ex = exp(-0.5*sq), in place over sq
                    nc.scalar.activation(sq[:], sq[:], ACTF.Exp, scale=-0.5)
                    # co = cos(t) = sin(t + pi/2), in place over diff
                    nc.scalar.activation(dflat, dflat, ACTF.Sin,
                                         bias=sct[:, 2:3], scale=sct[:, 0:1])
                    # co2 = co*(1-blend) on gpsimd, comb = ex*blend + co2
                    co2 = pool.tile([128, KT, fn], BF16, tag="co2", bufs=1)
                    c2f = co2[:].rearrange("p k j -> p (k j)")
                    nc.gpsimd.tensor_scalar_mul(c2f, dflat, sct[:, 3:4])
                    nc.vector.scalar_tensor_tensor(
                        c2f, sq[:], sct[:, 1:2], c2f, ALU.mult, ALU.add,
                    )
                    comb = co2[:]

                    pe = pool.tile([128, KT, OD], BF16, tag="pe", bufs=1)
                    for (srcs, dsts, ln) in runs:
                        nc.scalar.copy(
                            pe[:, :, dsts:dsts + ln], comb[:, :, srcs:srcs + ln]
                        )

                    wt = pool.tile([128, KT, OD], BF16, tag="wt", bufs=1)
                    nc.vector.tensor_tensor(
                        wt[:, :, 0:C], Atc[:], pe[:, :, 0:C], ALU.add)
                    a3, b3 = _bcast(pe[:, :, C:OD], fsb)
                    nc.vector.tensor_tensor(wt[:, :, C:OD], a3, b3, ALU.add)
                    nc.vector.tensor_tensor(wt[:], wt[:], pe[:], ALU.mult)

                    wv = wt[:].rearrange("p k c -> p c k")
                    if kc == 0:
                        nc.vector.tensor_reduce(
                            wsum[:], wv, mybir.AxisListType.X, ALU.add
                        )
                        nc.vector.tensor_reduce(
                            wmax[:], wv, mybir.AxisListType.X, ALU.max
                        )
                    else:
                        prs = pool.tile([128, OD], F32, tag="prs")
                        nc.vector.tensor_reduce(
                            prs[:], wv, mybir.AxisListType.X, ALU.add
                        )
                        nc.vector.tensor_tensor(wsum[:], wsum[:], prs[:], ALU.add)
                        prm = pool.tile([128, OD], F32, tag="prm")
                        nc.vector.tensor_reduce(
                            prm[:], wv, mybir.AxisListType.X, ALU.max
                        )
                        nc.vector.tensor_tensor(wmax[:], wmax[:], prm[:], ALU.max)

                # feat = gelu(wsum/K + wmax)
                ft = pool.tile([128, OD], F32, tag="ft")
                nc.vector.scalar_tensor_tensor(
                    ft[:], wsum[:], 1.0 / K, wmax[:], ALU.mult, ALU.add
                )
                fo = pool.tile([128, OD], F32, tag="fo")
                nc.scalar.activation(fo[:], ft[:], ACTF.Gelu)
                nc.gpsimd.dma_start(outf[r0:r0 + 128], fo[:])

                if first:
                    nc.vector.tensor_copy(accs[b][:], fo[:])
                    nc.vector.tensor_copy(accm[b][:], fo[:])
                else:
                    nc.vector.tensor_tensor(accs[b][:], accs[b][:], fo[:], ALU.add)
                    nc.vector.tensor_tensor(accm[b][:], accm[b][:], fo[:], ALU.max)

            # cross-partition (over S) reductions + output pieces
            for b in range(BL):
                s_, m_ = accs[b], accm[b]
                rs = pool.tile([128, OD], F32, tag="rs", bufs=2)
                nc.gpsimd.partition_all_reduce(
                    rs[:], s_[:], 128, bass_isa.ReduceOp.add)
                rm = pool.tile([128, OD], F32, tag="rm", bufs=2)
                nc.gpsimd.partition_all_reduce(
                    rm[:], m_[:], 128, bass_isa.ReduceOp.max)
                po = pool.tile([1, 2 * OD], F32, tag="po", bufs=2)
                nc.vector.tensor_copy(po[0:1, 0:OD], rm[0:1, :])
                nc.vector.tensor_scalar_mul(po[0:1, OD:2 * OD], rs[0:1, :], 1.0 / S)
                nc.sync.dma_start(outp[b:b + 1, :], po[:])
    nc.finalize()
    return nc


# ----------------------------------------------------------------------------
# orchestration
# ----------------------------------------------------------------------------

_CACHE = {}


def _graphs():
    if "g" not in _CACHE:
        _CACHE["g"] = (build_phase0(),
                       [build_stage(S, OD) for S, OD in STAGES])
    return _CACHE["g"]


def _run(nc, in_maps, tag=""):
    global LAST_EXEC_NS
    import tempfile
    import time
    t0 = time.perf_counter()
    res = bass_utils.run_bass_kernel_spmd(
        nc, in_maps, core_ids=list(range(NCORES)))
    dt = time.perf_counter() - t0
    if TRACE:
        PROFILES.append((tag, "", int(dt * 1e9)))
    ns = getattr(res, "exec_time_ns", None)
    LAST_EXEC_NS += int(ns) if ns else int(dt * 1e9)
    return res.results


def kernel(xyz):
    global LAST_EXEC_NS
    LAST_EXEC_NS = 0
    xyz = np.ascontiguousarray(np.asarray(xyz, np.float32))  # [32, 2048, 3]
    nc0, stage_ncs = _graphs()

    # ---- phase 0: initial adaptive embedding
    fd0, fn0, oi0, fv0, asig0, blend0 = _emb_params(xyz, INIT_DIM)
    inv_asig0 = 1.0 / (asig0 + EPS)
    fvrep0 = np.ascontiguousarray(np.tile(fv0, (128, 3)))
    screp0 = np.ascontiguousarray(
        np.tile(np.array([inv_asig0, blend0, np.pi / 2, 1.0 - blend0], np.float32), (128, 1)))
    in_maps = []
    for c in range(NCORES):
        xs = np.ascontiguousarray(
            xyz[c * BL:(c + 1) * BL].reshape(BL * N, 3))
        in_maps.append({"xyz": xs, "fv": fvrep0, "sc": screp0})
    arB = np.arange(B)

    def _geometry(cur_xyz, S, OD):
        # xyz-only host work for one stage: fps, knn, normalized xyz_knn,
        # and the embedding scalars.  Independent of device results, so it
        # can overlap with the previous stage's device execution.
        fps_idx = _fps(cur_xyz, S)                       # [B,S]
        xyz_s = cur_xyz[arB[:, None], fps_idx]           # [B,S,3]
        knn = _knn_idx(xyz_s, cur_xyz)                   # [B,S,K]
        xyz_knn = cur_xyz[arB[:, None, None], knn]       # [B,S,K,3]
        d = xyz_knn - xyz_s[:, :, None, :]
        stdx = np.clip(d.std(axis=(0, 1, 3), ddof=1), 1e-5, None)  # [K]
        xnn = d / stdx[None, None, :, None]
        fd, fn, oi, fvv, asig, blend = _emb_params(
            xnn.reshape(B, S * K, 3), OD)
        return fps_idx, knn, xnn, fvv, asig, blend

    import concurrent.futures
    pool = concurrent.futures.ThreadPoolExecutor(max_workers=1)
    pieces = []
    # stage-1 geometry needs only xyz — overlap it with the phase-0 call
    geo0_fut = pool.submit(_geometry, xyz, STAGES[0][0], STAGES[0][1])
    res = _run(nc0, in_maps, tag="p0")
    feat = np.concatenate(
        [np.asarray(res[c]["out"]).reshape(BL, N, INIT_DIM)
         for c in range(NCORES)], axis=0)
    geo = geo0_fut.result()
    cur_xyz = xyz
    for si, (S, OD) in enumerate(STAGES):
        C = OD // 2
        fps_idx, knn, xnn, fvv, asig, blend = geo
        xyz_s = cur_xyz[arB[:, None], fps_idx]           # [B,S,3]
        geo_fut = None
        if si + 1 < len(STAGES):
            geo_fut = pool.submit(_geometry, xyz_s,
                                  STAGES[si + 1][0], STAGES[si + 1][1])
        feat_s = feat[arB[:, None], fps_idx]             # [B,S,C]
        feat_knn = feat[arB[:, None, None], knn]         # [B,S,K,C]

        df = feat_knn - feat_s[:, :, None, :]
        stdf = np.clip(df.std(axis=(0, 1, 3), ddof=1), 1e-5, None)  # [K]

        inv_asig = 1.0 / (asig + EPS)

        fvrep = np.ascontiguousarray(np.tile(fvv, (128, 3)))
        screp = np.ascontiguousarray(
            np.tile(np.array([inv_asig, blend, np.pi / 2, 1.0 - blend], np.float32),
                    (128, 1)))
        isgrep = np.ascontiguousarray(
            np.tile((1.0 / stdf).astype(np.float32), (128, 1)))

        in_maps = []
        for c in range(NCORES):
            sl = slice(c * BL, (c + 1) * BL)
            in_maps.append({
                "araw": np.ascontiguousarray(
                    feat_knn[sl].reshape(BL * S, K, C).astype(_BF)),
                "fs": np.ascontiguousarray(
                    feat_s[sl].reshape(BL * S, C).astype(_BF)),
                "xn": np.ascontiguousarray(
                    xnn[sl].reshape(BL * S, K, 3).astype(_BF)),
                "fv": fvrep, "sc": screp, "isg": isgrep,
            })
        res = _run(stage_ncs[si], in_maps, tag='s%d' % (si+1))
        feat = np.concatenate(
            [np.asarray(res[c]["outf"]).astype(np.float32).reshape(BL, S, OD)
             for c in range(NCORES)], axis=0)
        piece = np.concatenate(
            [np.asarray(res[c]["outp"]) for c in range(NCORES)], axis=0)
        pieces.append(piece)
        cur_xyz = xyz_s
        if geo_fut is not None:
            geo = geo_fut.result()

    pool.shutdown(wait=False)
    return np.concatenate(pieces, axis=1).astype(np.float32)  # [B, 1920]



# revision 5
# speedup vs baseline: 27.7319x; 27.7319x over previous
"""Trainium2 Bass kernel for nn_AdaptiveEncoderCls_so (retrieval_knn).

Single fused device program across 8 NeuronCores (data-parallel over batch,
4 batch elements per core).  Host does only the xyz-side index math (furthest
point sampling + exact KNN, in C via cffi) and the xyz-side statistics, all
exact f32.  The device program computes the initial adaptive embedding, and
for each of the 4 encoder stages: gathers neighbor rows from a packed
[xyz_f32 | feat_bf16] DRAM table by indirect DMA (once, stashed to DRAM),
computes the global per-rank feature stds with a cross-core AllReduce,
then normalizes, embeds, aggregates, pools and gelus — writing the next
stage's table without ever returning features to the host.  Only the final
[4, 1920] pooled rows leave the device.
"""

import math
import sys

import numpy as np
import ml_dtypes

sys.path.insert(0, "/opt/trn_rl_repo")

import concourse.bass as bass  # noqa: E402
from concourse.bacc import Bacc  # noqa: E402
import concourse.mybir as mybir  # noqa: E402
from concourse.tile import TileContext  # noqa: E402
from concourse import bass_isa  # noqa: E402

F32 = mybir.dt.float32
BF16 = mybir.dt.bfloat16
I32 = mybir.dt.int32
ALU = mybir.AluOpType
ACTF = mybir.ActivationFunctionType

NCORES = 8
B, N, K = 32, 2048, 32
BL = B // NCORES
INIT_DIM = 32
SIGMA, BASELINE, SCALING, EPS = 0.26, 0.1, 10.0, 1e-6
STAGES = [(1024, 64), (512, 128), (256, 256), (128, 512)]  # (S, out_dim)
KT_BY_OD = {64: 32, 128: 32, 256: 32, 512: 16}

_BF = ml_dtypes.bfloat16
LAST_EXEC_NS = 0
TRACE = False
PROFILES = []
DEBUG_TABLES = False
LAST_RES = None

# ----------------------------------------------------------------------------
# C library: fps + knn (single-core container; numpy is too slow)
# ----------------------------------------------------------------------------

_CSRC = r"""
#include <math.h>

static float dbuf[4096];
static float xb0[4096], xb1[4096], xb2[4096];

void fps(const float* xyz, int Bb, int Nn, int npoint, int* out) {
    for (int b = 0; b < Bb; b++) {
        const float* x = xyz + (long)b * Nn * 3;
        int* o = out + (long)b * npoint;
        for (int i = 0; i < Nn; i++) {
            xb0[i] = x[i*3]; xb1[i] = x[i*3+1]; xb2[i] = x[i*3+2];
            dbuf[i] = 3.4e38f;
        }
        int far = 0;
        for (int it = 0; it < npoint; it++) {
            o[it] = far;
            float cx = xb0[far], cy = xb1[far], cz = xb2[far];
            for (int i = 0; i < Nn; i++) {
                float dx = xb0[i] - cx, dy = xb1[i] - cy, dz = xb2[i] - cz;
                float d = (dx*dx + dy*dy) + dz*dz;
                dbuf[i] = d < dbuf[i] ? d : dbuf[i];
            }
            float best = dbuf[0];
            for (int i = 1; i < Nn; i++)
                best = dbuf[i] > best ? dbuf[i] : best;
            int bi = 0;
            while (dbuf[bi] != best) bi++;
            far = bi;
        }
    }
}

#ifdef __AVX512F__
#include <immintrin.h>
#endif

static inline void knn_insert(float* vals, int* idxs, int* cnt, int Kk,
                              float* worst, float d, int m) {
    int c = *cnt;
    int lo = c;
    while (lo > 0 && vals[lo-1] > d) lo--;
    int end = c < Kk ? c : Kk - 1;
    for (int j = end; j > lo; j--) {
        vals[j] = vals[j-1]; idxs[j] = idxs[j-1];
    }
    vals[lo] = d; idxs[lo] = m;
    if (c < Kk) { c++; *cnt = c; }
    *worst = vals[c-1];
}

void knn(const float* xs, const float* x, int Bb, int S, int M, int Kk,
         int* out) {
    static float sqx[4096];
    for (int b = 0; b < Bb; b++) {
        const float* xb = x + (long)b * M * 3;
        const float* sb = xs + (long)b * S * 3;
        int* ob = out + (long)b * S * Kk;
        for (int m = 0; m < M; m++) {
            xb0[m] = xb[m*3]; xb1[m] = xb[m*3+1]; xb2[m] = xb[m*3+2];
            sqx[m] = xb0[m]*xb0[m] + xb1[m]*xb1[m] + xb2[m]*xb2[m];
        }
        for (int s = 0; s < S; s++) {
            float s0 = sb[s*3], s1 = sb[s*3+1], s2 = sb[s*3+2];
            float sq = s0*s0 + s1*s1 + s2*s2;
            for (int m = 0; m < M; m++) {
                float dot = s0*xb0[m] + s1*xb1[m] + s2*xb2[m];
                dbuf[m] = (-2.0f*dot + sq) + sqx[m];
            }
            float vals[64]; int idxs[64];
            int cnt = 0;
            float worst = 3.4e38f;
            int m0 = 0;
#ifdef __AVX512F__
            /* scalar seed until the top-K buffer is full */
            for (; m0 < M && cnt < Kk; m0++)
                knn_insert(vals, idxs, &cnt, Kk, &worst, dbuf[m0], m0);
            /* vector phase: skip 16 candidates at a time when none beat
               the current worst; process hits in ascending lane order */
            for (; m0 + 16 <= M; m0 += 16) {
                __m512 dv = _mm512_loadu_ps(dbuf + m0);
                __mmask16 mk = _mm512_cmp_ps_mask(
                    dv, _mm512_set1_ps(worst), _CMP_LT_OQ);
                while (mk) {
                    int lane = __builtin_ctz(mk);
                    mk &= mk - 1;
                    float d = dbuf[m0 + lane];
                    if (d < worst)
                        knn_insert(vals, idxs, &cnt, Kk, &worst, d, m0 + lane);
                }
            }
#endif
            for (; m0 < M; m0++) {
                float d = dbuf[m0];
                if (cnt == Kk && d >= worst) continue;
                knn_insert(vals, idxs, &cnt, Kk, &worst, d, m0);
            }
            for (int j = 0; j < Kk; j++) ob[s*Kk + j] = idxs[j];
        }
    }
}

void xstats(const float* x, const float* xs, const int* kn,
            int Bb, int S, int M, int Kk, double* s1, double* s2) {
    /* s1,s2: [Bb,3,Kk] sums of d and d*d over s, d = x[b,kn[b,s,k],c]-xs[b,s,c] */
    for (int b = 0; b < Bb; b++) {
        const float* xb = x + (long)b * M * 3;
        const float* sb = xs + (long)b * S * 3;
        const int* kb = kn + (long)b * S * Kk;
        double* s1b = s1 + (long)b * 3 * Kk;
        double* s2b = s2 + (long)b * 3 * Kk;
        for (int i = 0; i < 3 * Kk; i++) { s1b[i] = 0.0; s2b[i] = 0.0; }
        for (int s = 0; s < S; s++) {
            float c0 = sb[s*3], c1 = sb[s*3+1], c2 = sb[s*3+2];
            const int* kr = kb + (long)s * Kk;
            for (int k = 0; k < Kk; k++) {
                const float* p = xb + (long)kr[k] * 3;
                double d0 = (double)(p[0] - c0);
                double d1 = (double)(p[1] - c1);
                double d2 = (double)(p[2] - c2);
                s1b[0*Kk+k] += d0; s2b[0*Kk+k] += d0*d0;
                s1b[1*Kk+k] += d1; s2b[1*Kk+k] += d1*d1;
                s1b[2*Kk+k] += d2; s2b[2*Kk+k] += d2*d2;
            }
        }
    }
}
"""


_CLIB = None


def _get_clib():
    global _CLIB
    if _CLIB is not None:
        return _CLIB
    try:
        import cffi
        import tempfile
        ffi = cffi.FFI()
        ffi.cdef("void fps(const float*, int, int, int, int*);\n"
                 "void knn(const float*, const float*, int, int, int, int, int*);\n"
                 "void xstats(const float*, const float*, const int*, int, int, int, int, double*, double*);")
        d = tempfile.mkdtemp(prefix="aek_c_")
        ffi.set_source("_aek_c", _CSRC,
                       extra_compile_args=["-O3", "-ffp-contract=off",
                                           "-march=native"])
        ffi.compile(tmpdir=d, verbose=False)
        sys.path.insert(0, d)
        import _aek_c  # noqa
        _CLIB = (_aek_c.ffi, _aek_c.lib)
    except Exception:
        _CLIB = False
    return _CLIB


def _fps_np(xyz, npoint):
    Bb, Nn, _ = xyz.shape
    dist = np.full((Bb, Nn), np.inf, np.float32)
    far = np.zeros(Bb, np.int64)
    idxs = np.empty((Bb, npoint), np.int64)
    ar = np.arange(Bb)
    buf = np.empty_like(xyz)
    d = np.empty((Bb, Nn), np.float32)
    for i in range(npoint):
        idxs[:, i] = far
        c = xyz[ar, far]
        np.subtract(xyz, c[:, None, :], out=buf)
        np.multiply(buf, buf, out=buf)
        buf.sum(-1, out=d)
        np.minimum(dist, d, out=dist)
        far = dist.argmax(-1)
    return idxs.astype(np.int32)


def _knn_np(xyz_s, xyz, Kk):
    sq = -2.0 * np.matmul(xyz_s, xyz.transpose(0, 2, 1))
    sq += (xyz_s ** 2).sum(-1, dtype=np.float32)[:, :, None]
    sq += (xyz ** 2).sum(-1, dtype=np.float32)[:, None, :]
    M = min(Kk + 16, sq.shape[-1])
    if M >= sq.shape[-1]:
        return np.argsort(sq, axis=-1, kind="stable")[:, :, :Kk].astype(np.int32)
    part = np.argpartition(sq, M, axis=-1)[:, :, :M]
    vals = np.take_along_axis(sq, part, axis=-1)
    order = np.lexsort((part, vals), axis=-1)[:, :, :Kk]
    return np.take_along_axis(part, order, axis=-1).astype(np.int32)


def _fps(xyz, npoint):
    clib = _get_clib()
    if not clib:
        return _fps_np(xyz, npoint)
    ffi, lib = clib
    xyz = np.ascontiguousarray(xyz, np.float32)
    out = np.empty((xyz.shape[0], npoint), np.int32)
    lib.fps(ffi.cast("const float*", xyz.ctypes.data), xyz.shape[0],
            xyz.shape[1], npoint, ffi.cast("int*", out.ctypes.data))
    return out


def _knn(xyz_s, xyz):
    clib = _get_clib()
    if not clib:
        return _knn_np(xyz_s, xyz, K)
    ffi, lib = clib
    xyz_s = np.ascontiguousarray(xyz_s, np.float32)
    xyz = np.ascontiguousarray(xyz, np.float32)
    Bb, S = xyz_s.shape[0], xyz_s.shape[1]
    out = np.empty((Bb, S, K), np.int32)
    lib.knn(ffi.cast("const float*", xyz_s.ctypes.data),
            ffi.cast("const float*", xyz.ctypes.data),
            Bb, S, xyz.shape[1], K, ffi.cast("int*", out.ctypes.data))
    return out


def _xyz_stats(cur_xyz, xyz_s, knn, S):
    """stdx[K] and gstd for the stage, matching the reference's
    np.std(..., ddof=1) formulas (f64 accumulation in C)."""
    clib = _get_clib()
    if clib:
        ffi, lib = clib
        s1 = np.empty((B, 3, K), np.float64)
        s2 = np.empty((B, 3, K), np.float64)
        kn32 = np.ascontiguousarray(knn, np.int32)
        cx = np.ascontiguousarray(cur_xyz, np.float32)
        xs = np.ascontiguousarray(xyz_s, np.float32)
        lib.xstats(ffi.cast("const float*", cx.ctypes.data),
                   ffi.cast("const float*", xs.ctypes.data),
                   ffi.cast("const int*", kn32.ctypes.data),
                   B, S, cur_xyz.shape[1], K,
                   ffi.cast("double*", s1.ctypes.data),
                   ffi.cast("double*", s2.ctypes.data))
        n = B * S * 3
        S1 = s1.sum(axis=(0, 1))
        S2 = s2.sum(axis=(0, 1))
        var = (S2 - S1 * S1 / n) / (n - 1)
        stdx = np.clip(np.sqrt(np.maximum(var, 0.0)), 1e-5, None)
        n2 = S * K
        A = (s1 / stdx[None, None, :]).sum(-1)
        Q = (s2 / (stdx[None, None, :] ** 2)).sum(-1)
        var2 = (Q - A * A / n2) / (n2 - 1)
        gstd = float(np.mean(np.sqrt(np.maximum(var2, 0.0))))
        return stdx.astype(np.float32), gstd
    arB = np.arange(B)
    xyz_knn = cur_xyz[arB[:, None, None], knn]
    dd = xyz_knn - xyz_s[:, :, None, :]
    stdx = np.clip(dd.std(axis=(0, 1, 3), ddof=1), 1e-5, None)
    xnn = dd / stdx[None, None, :, None]
    gstd = float(np.mean(np.std(xnn.reshape(B, S * K, 3), axis=1, ddof=1)))
    return stdx.astype(np.float32), gstd


# ----------------------------------------------------------------------------
# host-side embedding params
# ----------------------------------------------------------------------------

def _emb_params(out_dim, gstd):
    fd = math.ceil(out_dim / 3)
    fn = fd * 3
    out_idx = np.floor(np.linspace(0, fn - 1, out_dim)).astype(np.int64)
    fv = np.linspace(-1.0, 1.0, fd + 2)[1:-1].astype(np.float32)
    asig = SIGMA * (1.0 + gstd)
    blend = float(1.0 / (1.0 + np.exp(-(gstd - BASELINE) * SCALING)))
    return fd, fn, out_idx, fv, float(asig), blend


def _runs(out_idx):
    runs = []
    s = 0
    n = len(out_idx)
    while s < n:
        e = s
        while e + 1 < n and out_idx[e + 1] == out_idx[e] + 1:
            e += 1
        runs.append((int(out_idx[s]), s, e - s + 1))
        s = e + 1
    return runs


def _bcast(a, b):
    return bass.broadcast_tensor_aps(a, b)


# ----------------------------------------------------------------------------
# the fused device program
# ----------------------------------------------------------------------------

def build_graph():
    nc = Bacc(num_devices=NCORES)
    fd0, fn0 = 11, 33
    runs0 = _runs(np.floor(np.linspace(0, fn0 - 1, INIT_DIM)).astype(np.int64))

    xyz_in = nc.dram_tensor("xyz", [BL * N, 3], F32, kind="ExternalInput")
    fv0_in = nc.dram_tensor("fv0", [128, 3 * fd0], F32, kind="ExternalInput")
    sc0_in = nc.dram_tensor("sc0", [128, 4], F32, kind="ExternalInput")
    out = nc.dram_tensor("out", [BL, 1920], F32, kind="ExternalOutput")

    stage_ins = []
    tkind = "ExternalOutput" if DEBUG_TABLES else "Internal"
    tables = [nc.dram_tensor("T0", [BL * N, 3 + INIT_DIM], F32,
                             kind=tkind)]
    for si, (S, OD) in enumerate(STAGES):
        C = OD // 2
        fd = math.ceil(OD / 3)
        TILES = BL * S // 128
        d = {
            "gidx": nc.dram_tensor(f"gidx{si}", [128, TILES * K], I32,
                                   kind="ExternalInput"),
            "fidx": nc.dram_tensor(f"fidx{si}", [128, TILES], I32,
                                   kind="ExternalInput"),
            "xyzs": nc.dram_tensor(f"xyzs{si}", [BL * S, 3], F32,
                                   kind="ExternalInput"),
            "fv": nc.dram_tensor(f"fvs{si}", [128, 3 * fd], F32,
                                 kind="ExternalInput"),
            "sc": nc.dram_tensor(f"scs{si}", [128, 4], F32,
                                 kind="ExternalInput"),
            "isgx": nc.dram_tensor(f"isgx{si}", [128, K], F32,
                                   kind="ExternalInput"),
            "stash": nc.dram_tensor(f"stash{si}", [BL * S, K * (3 + C)], F32,
                                    kind=tkind),
            "ccb": nc.dram_tensor(f"ccb{si}", [128, 2 * K], F32),
        }
        stage_ins.append(d)
        if si + 1 < len(STAGES):
            tables.append(nc.dram_tensor(f"T{si+1}", [BL * S, 3 + OD], F32,
                                         kind=tkind))

    with TileContext(nc) as tc:
        with tc.tile_pool(name="cst", bufs=1) as cpool, \
             tc.tile_pool(name="acc", bufs=1) as apool:

            # ---------------- phase 0: initial embedding + T0 ----------------
            PTS = BL * N // 128
            with tc.tile_pool(name="wrkp0", bufs=1) as pool:
                fvt = cpool.tile([128, 3 * fd0], F32, name="fv0t")
                nc.sync.dma_start(fvt[:], fv0_in[:])
                sct0 = cpool.tile([128, 4], F32, name="sc0t")
                nc.sync.dma_start(sct0[:], sc0_in[:])
                xt = pool.tile([128, PTS, 3], F32)
                nc.sync.dma_start(xt[:],
                                  xyz_in.rearrange("(p n) c -> p n c", p=128))
                diff0 = pool.tile([128, PTS, 3, fd0], F32)
                a4 = xt[:].unsqueeze(3)
                b4 = fvt[:].rearrange("p (c j) -> p c j", c=3).unsqueeze(1)
                a4, b4 = _bcast(a4, b4)
                nc.vector.tensor_tensor(diff0[:], a4, b4, ALU.subtract)
                dflat = diff0[:].rearrange("p n c j -> p (n c j)")
                sq0 = pool.tile([128, PTS * fn0], F32)
                nc.scalar.activation(sq0[:], dflat, ACTF.Square,
                                     scale=sct0[:, 0:1])
                nc.scalar.activation(sq0[:], sq0[:], ACTF.Exp, scale=-0.5)
                nc.scalar.activation(dflat, dflat, ACTF.Sin, bias=sct0[:, 2:3],
                                     scale=sct0[:, 0:1])
                dmc = pool.tile([128, PTS * fn0], F32)
                nc.vector.tensor_tensor(dmc[:], sq0[:], dflat, ALU.subtract)
                comb0 = pool.tile([128, PTS, fn0], F32)
                nc.vector.scalar_tensor_tensor(
                    comb0[:].rearrange("p n j -> p (n j)"),
                    dmc[:], sct0[:, 1:2], dflat, ALU.mult, ALU.add)
                t0v = tables[0].rearrange("(p n) e -> p n e", p=128)
                feat0 = pool.tile([128, PTS, INIT_DIM], F32)
                for (srcs, dsts, ln) in runs0:
                    nc.scalar.copy(feat0[:, :, dsts:dsts + ln],
                                   comb0[:, :, srcs:srcs + ln])
                nc.sync.dma_start(t0v[:, :, 3:3 + INIT_DIM], feat0[:])
                nc.gpsimd.dma_start(t0v[:, :, 0:3], xt[:])

            # ---------------- stages ----------------
            col0 = 0
            for si, (S, OD) in enumerate(STAGES):
                C = OD // 2
                fd = math.ceil(OD / 3)
                fn = 3 * fd
                runs = _runs(
                    np.floor(np.linspace(0, fn - 1, OD)).astype(np.int64))
                KT = KT_BY_OD[OD]
                NKT = K // KT
                TILES = BL * S // 128
                TPB = TILES // BL
                EP = 3 + C
                ins = stage_ins[si]
                Tprev = tables[si]
                nf = float(B * S * C)

                fvt = cpool.tile([128, 3 * fd], F32, name=f"fvt{si}")
                nc.sync.dma_start(fvt[:], ins["fv"][:])
                sct = cpool.tile([128, 4], F32, name=f"sct{si}")
                nc.sync.dma_start(sct[:], ins["sc"][:])
                isgxt = cpool.tile([128, K], F32, name=f"isgxt{si}")
                nc.sync.dma_start(isgxt[:], ins["isgx"][:])
                gidxt = cpool.tile([128, TILES * K], I32, name=f"git{si}")
                nc.sync.dma_start(gidxt[:], ins["gidx"][:])
                fidxt = cpool.tile([128, TILES], I32, name=f"fit{si}")
                nc.sync.dma_start(fidxt[:], ins["fidx"][:])

                isgf = apool.tile([128, K], F32, name=f"isgf{si}",
                                  tag=f"isgf{si}")
                fstiles = []
                stash_v = ins["stash"].rearrange("(t p) e -> t p e", p=128)

                # ---- pass 1: gather all neighbor rows once, stash them, and
                # accumulate per-rank feature-diff partial sums
                with tc.tile_pool(name=f"p1_{si}", bufs=1) as pool:
                    acc_s = pool.tile([128, K], F32, tag="accs")
                    acc_q = pool.tile([128, K], F32, tag="accq")
                    nc.vector.memset(acc_s[:], 0.0)
                    nc.vector.memset(acc_q[:], 0.0)
                    for ti in range(TILES):
                        fst = apool.tile([128, EP], F32, name=f"fs{si}_{ti}",
                                         tag=f"fs{si}_{ti}")
                        nc.gpsimd.indirect_dma_start(
                            out=fst[:], out_offset=None, in_=Tprev[:],
                            in_offset=bass.IndirectOffsetOnAxis(
                                ap=fidxt[:, ti:ti + 1], axis=0))
                        fstiles.append(fst)
                        gt = pool.tile([128, K * EP], F32, tag="gt",
                                       bufs=(2 if OD < 512 else 1))
                        for k in range(K):
                            nc.gpsimd.indirect_dma_start(
                                out=gt[:, k * EP:(k + 1) * EP],
                                out_offset=None,
                                in_=Tprev[:],
                                in_offset=bass.IndirectOffsetOnAxis(
                                    ap=gidxt[:, ti * K + k:ti * K + k + 1],
                                    axis=0))
                        nc.sync.dma_start(stash_v[ti], gt[:])
                        gtv = gt[:].rearrange("p (k e) -> p k e", k=K)
                        df = pool.tile([128, K, C], F32, tag="df", bufs=2)
                        a3, b3 = _bcast(gtv[:, :, 3:3 + C],
                                        fst[:, 3:3 + C].unsqueeze(1))
                        nc.vector.tensor_tensor(df[:], a3, b3, ALU.subtract)
                        r1 = pool.tile([128, K], F32, tag="r1", bufs=2)
                        nc.vector.tensor_reduce(r1[:], df[:],
                                                mybir.AxisListType.X, ALU.add)
                        nc.vector.tensor_tensor(acc_s[:], acc_s[:], r1[:],
                                                ALU.add)
                        dff = df[:].rearrange("p k c -> p (k c)")
                        nc.scalar.activation(dff, dff, ACTF.Square)
                        r2 = pool.tile([128, K], F32, tag="r2", bufs=2)
                        nc.vector.tensor_reduce(r2[:], df[:],
                                                mybir.AxisListType.X, ALU.add)
                        nc.vector.tensor_tensor(acc_q[:], acc_q[:], r2[:],
                                                ALU.add)

                    # core-local partition reduce + cross-core AllReduce
                    rs = pool.tile([128, K], F32, tag="rs")
                    nc.gpsimd.partition_all_reduce(rs[:], acc_s[:], 128,
                                                   bass_isa.ReduceOp.add)
                    rq = pool.tile([128, K], F32, tag="rq")
                    nc.gpsimd.partition_all_reduce(rq[:], acc_q[:], 128,
                                                   bass_isa.ReduceOp.add)
                    cct = pool.tile([128, 2 * K], F32, tag="cct")
                    nc.vector.tensor_copy(cct[:, 0:K], rs[:])
                    nc.vector.tensor_copy(cct[:, K:2 * K], rq[:])
                    nc.sync.dma_start(ins["ccb"][:], cct[:])
                    nc.gpsimd.collective_compute(
                        "AllReduce", ALU.add,
                        replica_groups=[list(range(NCORES))],
                        ins=[ins["ccb"][:]], outs=[ins["ccb"][:]])
                    ccr = pool.tile([128, 2 * K], F32, tag="ccr")
                    nc.sync.dma_start(ccr[:], ins["ccb"][:])
                    # isgf = min(rsqrt(max((q - s*s/n)/(n-1), 0)), 1e5)
                    mean = pool.tile([128, K], F32, tag="mean")
                    nc.vector.tensor_scalar_mul(mean[:], ccr[:, 0:K], 1.0 / nf)
                    var = pool.tile([128, K], F32, tag="var")
                    nc.vector.tensor_tensor(var[:], ccr[:, 0:K], mean[:],
                                            ALU.mult)
                    nc.vector.tensor_tensor(var[:], ccr[:, K:2 * K], var[:],
                                            ALU.subtract)
                    nc.vector.tensor_scalar(var[:], var[:],
                                            1.0 / (nf - 1.0), 0.0,
                                            ALU.mult, ALU.max)
                    stdt = pool.tile([128, K], F32, tag="stdt")
                    nc.scalar.activation(stdt[:], var[:], ACTF.Sqrt)
                    nc.vector.tensor_scalar_max(stdt[:], stdt[:], 1e-5)
                    nc.vector.reciprocal(isgf[:], stdt[:])

                # ---- pass 2: reload stash, normalize, embed, aggregate
                with tc.tile_pool(name=f"p2_{si}", bufs=1) as pool:
                    accs = []
                    accm = []
                    for bb in range(BL):
                        accs.append(pool.tile([128, OD], F32,
                                              name=f"pas{si}_{bb}",
                                              tag=f"accs{bb}"))
                        accm.append(pool.tile([128, OD], F32,
                                              name=f"pam{si}_{bb}",
                                              tag=f"accm{bb}"))
                    xyzs_v = ins["xyzs"].rearrange("(t p) c -> t p c", p=128)
                    if si + 1 < len(STAGES):
                        tnext_v = tables[si + 1].rearrange(
                            "(t p) e -> t p e", p=128)
                    for ti in range(TILES):
                        bb = ti // TPB
                        first = (ti % TPB) == 0
                        fst = fstiles[ti]
                        xst = pool.tile([128, 3], F32, tag="xst", bufs=2)
                        nc.sync.dma_start(xst[:], xyzs_v[ti])
                        gt = pool.tile([128, K * EP], F32, tag="g2",
                                       bufs=(2 if OD < 512 else 1))
                        nc.sync.dma_start(gt[:], stash_v[ti])
                        gtv = gt[:].rearrange("p (k e) -> p k e", k=K)
                        wsum = pool.tile([128, OD], F32, tag="wsum", bufs=2)
                        wmax = pool.tile([128, OD], F32, tag="wmax", bufs=2)
                        for kc in range(NKT):
                            ks = kc * KT
                            gts = gtv[:, ks:ks + KT, :]
                            Atc = pool.tile([128, KT, C], BF16, tag="Atc")
                            a3, b3 = _bcast(gts[:, :, 3:3 + C],
                                            fst[:, 3:3 + C].unsqueeze(1))
                            nc.vector.tensor_tensor(Atc[:], a3, b3,
                                                    ALU.subtract)
                            ig3 = isgf[:, ks:ks + KT].unsqueeze(2)
                            a3, b3 = _bcast(Atc[:], ig3)
                            nc.vector.tensor_tensor(Atc[:], a3, b3, ALU.mult)
                            xkn = gts[:, :, 0:3]
                            xnt = pool.tile([128, KT, 3], F32, tag="xnt")
                            a3, b3 = _bcast(xkn, xst[:].unsqueeze(1))
                            nc.vector.tensor_tensor(xnt[:], a3, b3,
                                                    ALU.subtract)
                            ig3x = isgxt[:, ks:ks + KT].unsqueeze(2)
                            a3, b3 = _bcast(xnt[:], ig3x)
                            nc.vector.tensor_tensor(xnt[:], a3, b3, ALU.mult)
                            diff = pool.tile([128, KT, 3, fd], BF16,
                                             tag="diff")
                            a4 = xnt[:].unsqueeze(3)
                            b4 = fvt[:].rearrange("p (c j) -> p c j", c=3)\
                                .unsqueeze(1)
                            a4, b4 = _bcast(a4, b4)
                            nc.vector.tensor_tensor(diff[:], a4, b4,
                                                    ALU.subtract)
                            dflat = diff[:].rearrange("p k c j -> p (k c j)")
                            sq = pool.tile([128, KT * fn], BF16, tag="sq")
                            nc.scalar.activation(sq[:], dflat, ACTF.Square,
                                                 scale=sct[:, 0:1])
                            nc.scalar.activation(sq[:], sq[:], ACTF.Exp,
                                                 scale=-0.5)
                            nc.scalar.activation(dflat, dflat, ACTF.Sin,
                                                 bias=sct[:, 2:3],
                                                 scale=sct[:, 0:1])
                            co2 = pool.tile([128, KT, fn], BF16, tag="co2")
                            c2f = co2[:].rearrange("p k j -> p (k j)")
                            nc.gpsimd.tensor_scalar_mul(c2f, dflat,
                                                        sct[:, 3:4])
                            nc.vector.scalar_tensor_tensor(
                                c2f, sq[:], sct[:, 1:2], c2f,
                                ALU.mult, ALU.add)
                            pe = pool.tile([128, KT, OD], BF16, tag="pe")
                            for (srcs, dsts, ln) in runs:
                                nc.scalar.copy(pe[:, :, dsts:dsts + ln],
                                               co2[:, :, srcs:srcs + ln])
                            wt = pool.tile([128, KT, OD], BF16, tag="wt")
                            nc.vector.tensor_tensor(wt[:, :, 0:C], Atc[:],
                                                    pe[:, :, 0:C], ALU.add)
                            a3, b3 = _bcast(pe[:, :, C:OD],
                                            fst[:, 3:3 + C].unsqueeze(1))
                            nc.vector.tensor_tensor(wt[:, :, C:OD], a3, b3,
                                                    ALU.add)
                            nc.vector.tensor_tensor(wt[:], wt[:], pe[:],
                                                    ALU.mult)
                            wv = wt[:].rearrange("p k c -> p c k")
                            if kc == 0:
                                nc.vector.tensor_reduce(
                                    wsum[:], wv, mybir.AxisListType.X, ALU.add)
                                nc.vector.tensor_reduce(
                                    wmax[:], wv, mybir.AxisListType.X, ALU.max)
                            else:
                                prs = pool.tile([128, OD], F32, tag="prs")
                                nc.vector.tensor_reduce(
                                    prs[:], wv, mybir.AxisListType.X, ALU.add)
                                nc.vector.tensor_tensor(wsum[:], wsum[:],
                                                        prs[:], ALU.add)
                                prm = pool.tile([128, OD], F32, tag="prm")
                                nc.vector.tensor_reduce(
                                    prm[:], wv, mybir.AxisListType.X, ALU.max)
                                nc.vector.tensor_tensor(wmax[:], wmax[:],
                                                        prm[:], ALU.max)

                        ft = pool.tile([128, OD], F32, tag="ft")
                        nc.vector.scalar_tensor_tensor(
                            ft[:], wsum[:], 1.0 / K, wmax[:], ALU.mult,
                            ALU.add)
                        fo = pool.tile([128, OD], F32, tag="fo", bufs=2)
                        nc.scalar.activation(fo[:], ft[:], ACTF.Gelu)
                        if si + 1 < len(STAGES):
                            nc.gpsimd.dma_start(tnext_v[ti, :, 3:3 + OD],
                                                fo[:])
                            nc.gpsimd.dma_start(tnext_v[ti, :, 0:3], xst[:])
                        if first:
                            nc.vector.tensor_copy(accs[bb][:], fo[:])
                            nc.vector.tensor_copy(accm[bb][:], fo[:])
                        else:
                            nc.vector.tensor_tensor(accs[bb][:], accs[bb][:],
                                                    fo[:], ALU.add)
                            nc.vector.tensor_tensor(accm[bb][:], accm[bb][:],
                                                    fo[:], ALU.max)

                    for bb in range(BL):
                        rs2 = pool.tile([128, OD], F32, tag="rs2", bufs=2)
                        nc.gpsimd.partition_all_reduce(
                            rs2[:], accs[bb][:], 128, bass_isa.ReduceOp.add)
                        rm2 = pool.tile([128, OD], F32, tag="rm2", bufs=2)
                        nc.gpsimd.partition_all_reduce(
                            rm2[:], accm[bb][:], 128, bass_isa.ReduceOp.max)
                        po = pool.tile([1, 2 * OD], F32, tag="po", bufs=2)
                        nc.vector.tensor_copy(po[0:1, 0:OD], rm2[0:1, :])
                        nc.vector.tensor_scalar_mul(po[0:1, OD:2 * OD],
                                                    rs2[0:1, :], 1.0 / S)
                        nc.sync.dma_start(
                            out[bb:bb + 1, col0:col0 + 2 * OD], po[:])
                col0 += 2 * OD
    nc.finalize()
    return nc


# ----------------------------------------------------------------------------
# cached-jit SPMD runner (inlined; avoids per-call retrace/recompile)
# ----------------------------------------------------------------------------

_RUNNER = {}


def _get_runner(nc):
    key = id(nc)
    if key in _RUNNER:
        return _RUNNER[key]
    import jax
    from jax.sharding import Mesh, PartitionSpec
    from jax.experimental.shard_map import shard_map
    from concourse.bass2jax import (_bass_exec_p, partition_id_tensor,
                                    install_neuronx_cc_hook)
    install_neuronx_cc_hook()
    partition_name = (nc.partition_id_tensor.name
                      if nc.partition_id_tensor else None)
    in_names, out_names, out_avals, zero_shapes = [], [], [], []
    for alloc in nc.m.functions[0].allocations:
        if not isinstance(alloc, mybir.MemoryLocationSet):
            continue
        name = alloc.memorylocations[0].name
        if alloc.kind == "ExternalInput":
            if name != partition_name:
                in_names.append(name)
        elif alloc.kind == "ExternalOutput":
            out_names.append(name)
            shape = tuple(alloc.tensor_shape)
            dtype = mybir.dt.np(alloc.dtype)
            out_avals.append(jax.core.ShapedArray(shape, dtype))
            zero_shapes.append((shape, dtype))
    n_params = len(in_names)
    n_outs = len(out_avals)
    all_in = list(in_names) + list(out_names)
    if partition_name is not None:
        all_in.append(partition_name)
    donate = tuple(range(n_params, n_params + n_outs))

    def _body(*args):
        operands = list(args)
        if partition_name is not None:
            operands.append(partition_id_tensor())
        return tuple(_bass_exec_p.bind(
            *operands, out_avals=tuple(out_avals), in_names=tuple(all_in),
            out_names=tuple(out_names),
            lowering_input_output_aliases=(),
            sim_require_finite=True, sim_require_nnan=True, nc=nc))

    devices = jax.devices()[:NCORES]
    mesh = Mesh(np.asarray(devices), ("core",))
    sharded = jax.jit(
        shard_map(_body, mesh=mesh,
                  in_specs=(PartitionSpec("core"),) * (n_params + n_outs),
                  out_specs=(PartitionSpec("core"),) * n_outs,
                  check_rep=False),
        donate_argnums=donate, keep_unused=True)
    r = (sharded, in_names, out_names, out_avals, zero_shapes)
    _RUNNER[key] = r
    return r


def _run_spmd(nc, in_maps):
    global LAST_EXEC_NS
    import time
    t0 = time.perf_counter()
    sharded, in_names, out_names, out_avals, zero_shapes = _get_runner(nc)
    concat_in = [
        np.concatenate([np.asarray(m[name]) for m in in_maps], axis=0)
        for name in in_names]
    concat_zeros = [np.zeros((NCORES * s[0], *s[1:]), d)
                    for (s, d) in zero_shapes]
    out_arrs = sharded(*concat_in, *concat_zeros)
    res = [
        {name: np.asarray(out_arrs[i]).reshape(NCORES, *out_avals[i].shape)[c]
         for i, name in enumerate(out_names)}
        for c in range(NCORES)]
    dt = time.perf_counter() - t0
    if TRACE:
        PROFILES.append(("fused", "", int(dt * 1e9)))
    LAST_EXEC_NS += int(dt * 1e9)
    return res


_GRAPH = {}


def _graph():
    if "g" not in _GRAPH:
        _GRAPH["g"] = build_graph()
    return _GRAPH["g"]


# ----------------------------------------------------------------------------
# kernel entry
# ----------------------------------------------------------------------------

def kernel(xyz):
    global LAST_EXEC_NS
    LAST_EXEC_NS = 0
    xyz = np.ascontiguousarray(np.asarray(xyz, np.float32))
    nc = _graph()
    arB = np.arange(B)

    gstd0 = float(np.mean(np.std(xyz, axis=1, ddof=1)))
    _, _, _, fv0, asig0, blend0 = _emb_params(INIT_DIM, gstd0)
    sc0 = np.tile(np.array([1.0 / (asig0 + EPS), blend0, np.pi / 2,
                            1.0 - blend0], np.float32), (128, 1))
    fv0rep = np.tile(np.tile(fv0, 3), (128, 1)).astype(np.float32)

    per_core = [{"xyz": np.ascontiguousarray(
        xyz[c * BL:(c + 1) * BL].reshape(BL * N, 3)),
        "fv0": fv0rep, "sc0": sc0} for c in range(NCORES)]

    cur_xyz = xyz
    M = N
    for si, (S, OD) in enumerate(STAGES):
        fps_idx = _fps(cur_xyz, S)                    # [B,S] int32
        xyz_s = cur_xyz[arB[:, None], fps_idx]        # [B,S,3]
        knn = _knn(xyz_s, cur_xyz)                    # [B,S,K] int32
        stdx, gstd = _xyz_stats(cur_xyz, xyz_s, knn, S)
        _, _, _, fvv, asig, blend = _emb_params(OD, gstd)

        TILES = BL * S // 128
        TPB = TILES // BL
        fvrep = np.tile(np.tile(fvv, 3), (128, 1)).astype(np.float32)
        screp = np.tile(np.array(
            [1.0 / (asig + EPS), blend, np.pi / 2, 1.0 - blend],
            np.float32), (128, 1))
        isgxrep = np.tile((1.0 / stdx).astype(np.float32), (128, 1))
        for c in range(NCORES):
            gb = c * BL + (np.arange(TILES) // TPB)
            lb = (np.arange(TILES) // TPB)
            idx = np.empty((128, TILES * K), np.int32)
            fidx = np.empty((128, TILES), np.int32)
            for ti in range(TILES):
                sp = (ti % TPB) * 128 + np.arange(128)
                idx[:, ti * K:(ti + 1) * K] = knn[gb[ti], sp, :] + lb[ti] * M
                fidx[:, ti] = fps_idx[gb[ti], sp] + lb[ti] * M
            per_core[c][f"gidx{si}"] = idx
            per_core[c][f"fidx{si}"] = fidx
            per_core[c][f"xyzs{si}"] = np.ascontiguousarray(
                xyz_s[c * BL:(c + 1) * BL].reshape(BL * S, 3))
            per_core[c][f"fvs{si}"] = fvrep
            per_core[c][f"scs{si}"] = screp
            per_core[c][f"isgx{si}"] = isgxrep
        cur_xyz = xyz_s
        M = S

    res = _run_spmd(nc, per_core)
    global LAST_RES
    LAST_RES = res
    return np.concatenate([res[c]["out"] for c in range(NCORES)],
                          axis=0).astype(np.float32)
